# revision 1
# baseline (speedup 1.0000x reference)
"""Trainium2 Bass kernel for nn_DGraFormer_framework (gnn_message_passing).

Reference computation (B=32, N=64, S=336, D=32, K=3 layers, beta=0.05):
    per (b, s):  A = adj[b,s]  (row-normalized [N,N])
    H0 = x w_start + b_start          [N, D]
    H_{k+1} = beta*x + (1-beta) A^T H_k
    out = concat(H_0..H_3) @ w_mlp + b_mlp   -> [b, n, s]

Everything is linear in the feature dim, so D collapses:
    out[b,:,s] = pre0 + A'(pre1 + A'(pre2 + A' pre3))      (Horner)
where A' = A^T and pre_j[b,n,s] = c_j * x[b,n,s] + d_j (scalars c_j, d_j, e
derived from w_start/b_start/w_mlp/b_mlp on the host; e folded into pre0).

Device kernel (per core; data-parallel over batch, 4 b per core):
  - adj[b] (336 s-slices of [64,64]) packed as 84 "quads": 4 A-matrices per
    128x128 stationary tile (2x2 blocks of 64x64), fp16.
  - 3 passes; pass k: one matmul per quad, moving operand [128,4] whose col
    4q+j carries chain s=4q+j's vector in one 64-partition half (zeros in the
    other).  Quad block (pb,cb) holds A_{s=4q+sigma(pb,cb)},
    sigma = [[1,0],[2,3]][pb][cb] (cb=0: s=4q+1+pb; cb=1: s=4q+3*pb), so
    col j classes: j0:(0,1) in-top/out-bot, j1:(0,0) top/top,
    j2:(1,0) in-bot/out-top, j3:(1,1) bot/bot.
  - Transitions between passes are batched strided DVE adds (psum + pre -> V);
    the two "crossed" classes (j0, j2) bounce through a shift-matmul whose
    stationary is the 64<->64 partition block swap matrix.
  - Final pass adds pre0 and lands all four classes in a contiguous [64,336]
    output tile (col order == s order), one DMA per b back to HBM.
"""

import sys

sys.path.insert(0, "/opt/trn_rl_repo")

import numpy as np

import concourse.bass as bass
import concourse.mybir as mybir
import concourse.tile as tile
from concourse import bacc
from concourse.bass_utils import run_bass_kernel_spmd

B, N, S, D = 32, 64, 336, 32
MP_LAYERS = 3
PROPBETA = 0.05
NCORES = 8
BL = B // NCORES          # batches per core
Q = S // 4                # quads per batch (84)

ADJ_DT = mybir.dt.float16     # quantized adjacency + chain-vector dtype
ADJ_NP = np.float16

f32 = mybir.dt.float32
COLT = 1                      # column-tiling split per quad matmul


def _coefficients(w_start, b_start, w_mlp, b_mlp):
    """Collapse the feature dim: out = sum_j A'^j (c_j x + d_j 1) + e (j=0..K).

    H_k = sum_j A'^j (x u_{k,j}^T + 1 v_{k,j}^T) with
    H_0: u=w_start, v=b_start;  H_{k+1} = beta x 1^T + (1-beta) A' H_k.
    """
    K = MP_LAYERS
    beta, sb = PROPBETA, 1.0 - PROPBETA
    ws = w_start[0].astype(np.float64)
    bs = b_start.astype(np.float64)
    w = [w_mlp[k * D:(k + 1) * D, 0].astype(np.float64) for k in range(K + 1)]

    u = {(0, 0): ws}
    v = {(0, 0): bs}
    for k in range(K):
        nu = {(k + 1, 0): beta * np.ones(D)}
        nv = {(k + 1, 0): np.zeros(D)}
        for j in range(k + 1):
            nu[(k + 1, j + 1)] = sb * u[(k, j)]
            nv[(k + 1, j + 1)] = sb * v[(k, j)]
        u.update(nu)
        v.update(nv)

    c = np.zeros(K + 1)
    d = np.zeros(K + 1)
    for k in range(K + 1):
        for j in range(k + 1):
            c[j] += float(u[(k, j)] @ w[k])
            d[j] += float(v[(k, j)] @ w[k])
    e = d[0] + float(b_mlp[0])
    return c, d, e


def _shift_matrix():
    sh = np.zeros((128, 128), dtype=np.float32)
    idx = np.arange(64)
    sh[idx, idx + 64] = 1.0
    sh[idx + 64, idx] = 1.0
    return sh


def _qview(ap):
    """[P, S] -> [P, q, f] with f in 0..3 (col = 4q+f)."""
    return ap.rearrange("p (q f) -> p q f", f=4)


def build_nc():
    nc = bacc.Bacc("TRN2", target_bir_lowering=False, debug=False)

    # adj pre-packed on host into the quad layout, fp16:
    # adjq[b, p, q*128 + cb*64 + m] = adj[b, 4q + sigma(pb,cb), n, m],
    # p = 64*pb + n, sigma = [[1,0],[2,3]][pb][cb]
    adj_l = nc.dram_tensor("adj", [BL, 128, Q * 128], ADJ_DT, kind="ExternalInput")
    pre_l = nc.dram_tensor("pre", [BL, MP_LAYERS + 1, N, S], f32,
                           kind="ExternalInput")
    shift16 = nc.dram_tensor("shift16", [128, 128], ADJ_DT, kind="ExternalInput")
    shift32 = nc.dram_tensor("shift32", [128, 128], f32, kind="ExternalInput")
    out_l = nc.dram_tensor("out", [BL, N, S], f32, kind="ExternalOutput")

    with tile.TileContext(nc) as tc:
        with (
            tc.tile_pool(name="singles", bufs=1) as singles,
            tc.tile_pool(name="adj_pool", bufs=2) as adj_pool,
            tc.tile_pool(name="pre_pool", bufs=2) as pre_pool,
            tc.tile_pool(name="o_pool", bufs=2) as o_pool,
            tc.tile_pool(name="ps_pool", bufs=6, space=bass.MemorySpace.PSUM)
            as ps_pool,
            tc.tile_pool(name="sh_pool", bufs=2, space=bass.MemorySpace.PSUM)
            as sh_pool,
        ):
            sh16 = singles.tile([128, 128], ADJ_DT, tag="sh16", name="sh16")
            nc.sync.dma_start(sh16[:], shift16[:])
            sh32 = singles.tile([128, 128], f32, tag="sh32", name="sh32")
            nc.sync.dma_start(sh32[:], shift32[:])

            # chain-vector tiles, double-buffered by batch parity so
            # consecutive batches pipeline; complementary halves stay zero.
            # Staging tiles feed shift matmuls that read both halves of
            # their columns; zero the never-written halves once.
            V = {}
            T16 = {}
            T32 = {}
            for par in (0, 1):
                for k in (1, 2, 3):
                    V[par, k] = singles.tile([128, S], ADJ_DT,
                                             tag=f"v{par}{k}", name=f"v{par}{k}")
                    nc.gpsimd.memset(V[par, k][:], 0.0)
                T16[par] = singles.tile([128, S], ADJ_DT,
                                        tag=f"t16_{par}", name=f"t16_{par}")
                nc.gpsimd.memset(T16[par][:], 0.0)
                T32[par] = singles.tile([128, S], f32,
                                        tag=f"t32_{par}", name=f"t32_{par}")
                nc.gpsimd.memset(T32[par][:], 0.0)

            NG = 2                       # quad groups per batch (pipelining)
            GQ = Q // NG                 # quads per group

            def pre_view(pre_t, lo, hi, j, gr, fsl):
                v = pre_t[lo:hi, :].rearrange("p (j q f) -> p j q f",
                                              j=MP_LAYERS + 1, f=4)
                return v[:, j, gr * GQ:(gr + 1) * GQ, fsl]

            def init_v3(par, pre_t):
                # V3 = pre3 at each chain's input half (j0,j1 top; j2,j3 bot)
                v3 = _qview(V[par, 3][:, :])
                for gr in range(NG):
                    cs = slice(gr * GQ, (gr + 1) * GQ)
                    nc.scalar.copy(v3[0:64, cs, 0:2],
                                   pre_view(pre_t, 0, 64, 3, gr, slice(0, 2)))
                    nc.scalar.copy(v3[64:128, cs, 2:4],
                                   pre_view(pre_t, 64, 128, 3, gr, slice(2, 4)))

            pre_tiles = {}
            finals = []
            for b in range(BL):
                par = b % 2
                # ---- pre vectors [128, 4*S], mirrored into both halves ----
                if b in pre_tiles:
                    pre_t = pre_tiles[b]
                else:
                    pre_t = pre_pool.tile([128, (MP_LAYERS + 1) * S], f32,
                                          tag="pre")
                    pre_tiles[b] = pre_t
                    src_pre = bass.AP(pre_l, b * (MP_LAYERS + 1) * N * S,
                                      [[S, N], [N * S, MP_LAYERS + 1], [1, S]])
                    nc.scalar.dma_start(out=pre_t[0:64, :], in_=src_pre)
                    nc.scalar.dma_start(out=pre_t[64:128, :], in_=src_pre)
                    init_v3(par, pre_t)

                # ---- load adj[b]: one DMA per quad-group ----
                adjq = adj_pool.tile([128, Q * 128], ADJ_DT, tag="adjq")
                half = Q * 128 // 2
                nc.sync.dma_start(out=adjq[:, 0:half], in_=adj_l[b][:, 0:half])
                nc.sync.dma_start(out=adjq[:, half:], in_=adj_l[b][:, half:])

                # ---- three matmul passes with group-pipelined transitions ----
                psum = {}
                for k in (3, 2, 1):
                    for gr in range(NG):
                        ps = ps_pool.tile([128, 4 * GQ], f32, tag="pass")
                        psum[k, gr] = ps
                        for q in range(gr * GQ, (gr + 1) * GQ):
                            lq = q - gr * GQ
                            nc.tensor.matmul(
                                ps[:, 4 * lq:4 * lq + 4],
                                adjq[:, 128 * q:128 * (q + 1)],
                                V[par, k][:, 4 * q:4 * q + 4],
                                start=True, stop=True,
                            )
                    if k == 3 and finals:
                        finals.pop(0)()   # previous batch's deferred epilogue
                    if k == 3 and b + 1 < BL:
                        # prefetch next batch's pre + V3 while this batch runs
                        nb = b + 1
                        npar = nb % 2
                        pre_n = pre_pool.tile([128, (MP_LAYERS + 1) * S], f32,
                                              tag="pre")
                        pre_tiles[nb] = pre_n
                        src_pre_n = bass.AP(
                            pre_l, nb * (MP_LAYERS + 1) * N * S,
                            [[S, N], [N * S, MP_LAYERS + 1], [1, S]])
                        nc.scalar.dma_start(out=pre_n[0:64, :], in_=src_pre_n)
                        nc.scalar.dma_start(out=pre_n[64:128, :], in_=src_pre_n)
                        init_v3(npar, pre_n)
                    if k > 1:
                        for gr in range(NG):
                            ps = psum[k, gr]
                            p = _qview(ps[:, :])
                            vn = _qview(V[par, k - 1][:, :])
                            cs = slice(gr * GQ, (gr + 1) * GQ)
                            tq = _qview(T16[par][:, :])
                            # crossed classes stage at their output half
                            nc.vector.tensor_add(
                                tq[0:64, cs, 2:3], p[0:64, :, 2:3],
                                pre_view(pre_t, 0, 64, k - 1, gr, slice(2, 3)))
                            nc.vector.tensor_add(
                                tq[64:128, cs, 0:1], p[64:128, :, 0:1],
                                pre_view(pre_t, 64, 128, k - 1, gr, slice(0, 1)))
                            # swap halves: moving cols {4q+0, 4q+2}
                            shp = sh_pool.tile([128, 2 * GQ], f32, tag="shift")
                            nc.tensor.matmul(shp[:, :], sh16[:],
                                             tq[:, cs, 0:3:2],
                                             start=True, stop=True)
                            # diagonal classes: direct adds
                            nc.vector.tensor_add(
                                vn[0:64, cs, 1:2], p[0:64, :, 1:2],
                                pre_view(pre_t, 0, 64, k - 1, gr, slice(1, 2)))
                            nc.vector.tensor_add(
                                vn[64:128, cs, 3:4], p[64:128, :, 3:4],
                                pre_view(pre_t, 64, 128, k - 1, gr, slice(3, 4)))
                            # copy swapped results back (ScalarE; PSUM-read ok)
                            g = shp[:, :].rearrange("p (q g) -> p q g", g=2)
                            nc.scalar.copy(vn[0:64, cs, 0:1], g[0:64, :, 0:1])
                            nc.scalar.copy(vn[64:128, cs, 2:3], g[64:128, :, 1:2])

                # ---- final: psum-reading adds now (frees PSUM); the
                # shift matmul + copy + output DMA are deferred past the
                # next batch's pass-3 to stay off the critical path
                O = o_pool.tile([64, S], f32, tag="o", name=f"o{b}")
                for gr in range(NG):
                    p1 = _qview(psum[1, gr][:, :])
                    ov = _qview(O[:, :])
                    t2 = _qview(T32[par][:, :])
                    cs = slice(gr * GQ, (gr + 1) * GQ)
                    # j0, j3 land on bottom: stage for the swap
                    nc.vector.tensor_add(
                        t2[64:128, cs, 0:4:3], p1[64:128, :, 0:4:3],
                        pre_view(pre_t, 64, 128, 0, gr, slice(0, 4, 3)))
                    # j1, j2 land on top directly
                    nc.vector.tensor_add(
                        ov[:, cs, 1:3], p1[0:64, :, 1:3],
                        pre_view(pre_t, 0, 64, 0, gr, slice(1, 3)))

                def make_final(b, par, O):
                    def emit():
                        for gr in range(NG):
                            ov = _qview(O[:, :])
                            t2 = _qview(T32[par][:, :])
                            cs = slice(gr * GQ, (gr + 1) * GQ)
                            shf = sh_pool.tile([128, 2 * GQ], f32, tag="shift")
                            nc.tensor.matmul(shf[:, :], sh32[:],
                                             t2[:, cs, 0:4:3],
                                             start=True, stop=True)
                            gf = shf[:, :].rearrange("p (q g) -> p q g", g=2)
                            nc.scalar.copy(ov[:, cs, 0:4:3], gf[0:64, :, :])
                        nc.sync.dma_start(out=out_l[b], in_=O[:])
                    return emit
                finals.append(make_final(b, par, O))
            for f in finals:
                f()

    nc.finalize()
    return nc


_NC_CACHE = None


def _get_nc():
    global _NC_CACHE
    if _NC_CACHE is None:
        _NC_CACHE = build_nc()
    return _NC_CACHE


def _pack_adj(adj):
    """[B, S, N, N] f32 -> [B, 128, Q*128] f16 quad layout (see build_nc)."""
    sigma = np.array([[1, 0], [2, 3]])  # [pb][cb]
    # s_idx[q, pb, cb] = 4q + sigma[pb, cb]
    s_idx = 4 * np.arange(Q)[:, None, None] + sigma[None, :, :]
    a = adj[:, s_idx]                      # [B, Q, 2pb, 2cb, n, m]
    a = a.transpose(0, 2, 4, 1, 3, 5)      # [B, pb, n, Q, cb, m]
    return np.ascontiguousarray(
        a.reshape(B, 128, Q * 128).astype(ADJ_NP))


def _prepare_in_maps(x, adj, w_start, b_start, w_mlp, b_mlp):
    c, d, e = _coefficients(np.asarray(w_start), np.asarray(b_start),
                            np.asarray(w_mlp), np.asarray(b_mlp))
    x = np.ascontiguousarray(np.asarray(x, dtype=np.float32))
    adj = _pack_adj(np.asarray(adj, dtype=np.float32))
    # pre[b, j, n, s] = c_j x + d_j (+ e for j=0)
    pre = np.empty((B, MP_LAYERS + 1, N, S), dtype=np.float32)
    for j in range(MP_LAYERS + 1):
        # e already folds in d[0] (+ b_mlp); j=0 must not add d[0] again
        pre[:, j] = c[j] * x + (e if j == 0 else d[j])
    sh = _shift_matrix()
    in_maps = []
    for i in range(NCORES):
        sl = slice(i * BL, (i + 1) * BL)
        in_maps.append({
            "adj": np.ascontiguousarray(adj[sl]),
            "pre": np.ascontiguousarray(pre[sl]),
            "shift16": sh.astype(ADJ_NP),
            "shift32": sh,
        })
    return in_maps


def run_spmd(inputs, trace=False, **kw):
    in_maps = _prepare_in_maps(**inputs)
    res = run_bass_kernel_spmd(_get_nc(), in_maps,
                               core_ids=list(range(NCORES)), trace=trace, **kw)
    out = np.concatenate([r["out"] for r in res.results], axis=0)
    return out, res


def kernel(**inputs):
    out, _ = run_spmd(inputs)
    return out.astype(np.float32)


if __name__ == "__main__":
    # quick smoke test against a numpy oracle
    rng = np.random.default_rng(0)
    x = rng.standard_normal((B, N, S), dtype=np.float32)
    adj = rng.random((B, S, N, N), dtype=np.float32)
    adj /= adj.sum(-1, keepdims=True)
    w_start = rng.standard_normal((1, D)).astype(np.float32)
    b_start = (rng.standard_normal(D) * 0.01).astype(np.float32)
    w_mlp = (rng.standard_normal(((MP_LAYERS + 1) * D, 1)) /
             np.sqrt((MP_LAYERS + 1) * D)).astype(np.float32)
    b_mlp = (rng.standard_normal(1) * 0.01).astype(np.float32)

    got = kernel(x=x, adj=adj, w_start=w_start, b_start=b_start,
                 w_mlp=w_mlp, b_mlp=b_mlp)

    h = x[..., None] * w_start[0] + b_start
    outs = [h]
    a = np.transpose(adj, (0, 2, 3, 1))
    for _ in range(MP_LAYERS):
        conv = np.einsum('bnsc,bnms->bmsc', h, a, optimize=True)
        h = PROPBETA * x[..., None] + (1 - PROPBETA) * conv
        outs.append(h)
    hc = np.concatenate(outs, axis=-1)
    want = (hc @ w_mlp)[..., 0] + b_mlp[0]

    aerr = np.abs(got - want)
    print("max abs err:", aerr.max(),
          "normalized:", aerr.max() / np.abs(want).max())

